# revision 7
# baseline (speedup 1.0000x reference)
"""Expert-router (MoE top-2 routing) Trainium2 Bass kernel, 8-core SPMD.

Reference computation (fp32):
  logits[b,s,e] = hidden_states[b,s,:] @ W[e,:] + b[e] + routing_bias[b,e]
  p = softmax(logits, axis=-1)
  top2_vals, top2_idx = top_k(p, 2)
  weights = top2_vals / (sum(top2_vals) + 1e-8)
returns (weights [4,4096,2] f32, indices [4,4096,2] int32).

Sharding (data/sequence parallel): the 4*4096 = 16384 tokens are split
contiguously across 8 NeuronCores (2048 tokens each; every core's range
falls inside a single batch, so its routing_bias row is folded into the
bias vector on the host). W^T and the bias are replicated.

Layout: hidden_states is transposed on the HOST into a d-major,
DMA-optimal layout ht5[tile, p, chunk, tok] = h[tok', 128*chunk+p]
(tok' = tile*TOKTILE + tok). This removes all on-device PE transposes
and their PSUM->SBUF copies (the baseline's dominant non-DMA cost) and
makes every DMA a 128-partition transfer with 16 KB contiguous
per-partition runs — the max-bandwidth descriptor shape.

Per-core kernel (all fp32; float32r would flip near-tie top-2 indices):
  - stream ht in [128, 32, TOKTILE] tiles (4 x 2 MB pieces, 8 d-chunks
    each, so matmuls start before the full tile lands)
  - matmul per 128-token group: PSUM[128 tok, 64 exp] accumulated over
    32 d-chunks, lhsT = ht chunk (stationary), rhs = W^T chunk (moving,
    64 cols -> cheapest PE orientation)
  - bias add fused with the PSUM->SBUF copy (DVE tensor_add, bias
    DMA-broadcast across partitions)
  - top-8 via DVE max/max_index; softmax via ACT exp with accumulated Z;
    renorm with exact DVE reciprocals:
        p1 = 1/Z, p2 = exp(l2-l1)/Z, w_i = p_i/(p1+p2+1e-8)
  - outputs accumulated in SBUF [128, 16, 2] and streamed out per tile
"""

from contextlib import ExitStack, nullcontext

import numpy as np

import concourse.mybir as mybir
import concourse.tile as tile
from concourse import bacc
from concourse.bass_utils import run_bass_kernel_spmd

P = 128
D = 4096
E = 64
NCHUNK = D // P          # 32
N_CORES = 8
B_DIM = 4
S = 4096                 # tokens per batch
T_TOTAL = B_DIM * S      # 16384
T_CORE = T_TOTAL // N_CORES  # 2048
TOKTILE = 512            # tokens per pipeline tile
NT = T_CORE // TOKTILE   # 4 tiles
GP = TOKTILE // P        # 4 groups per tile
NGRP = T_CORE // P       # 16
PIECES = 4               # DMA pieces per tile (8 chunks / 2 MB each)
EPS = 1e-8

f32 = mybir.dt.float32
i32 = mybir.dt.int32
u32 = mybir.dt.uint32

_NC_CACHE = None


def _build_nc(iters: int = 1, in_bufs: int = 2, pieces: int = PIECES,
              order: str = "g"):
    nc = bacc.Bacc("TRN2", target_bir_lowering=False, debug=False)

    ht5 = nc.dram_tensor(
        "ht5", [NT, P, NCHUNK, TOKTILE], f32, kind="ExternalInput"
    ).ap()
    wt = nc.dram_tensor("wt", [D, E], f32, kind="ExternalInput").ap()
    bias = nc.dram_tensor("bias", [1, E], f32, kind="ExternalInput").ap()
    w_out = nc.dram_tensor("w_out", [P, NGRP, 2], f32, kind="ExternalOutput").ap()
    i_out = nc.dram_tensor("i_out", [P, NGRP, 2], i32, kind="ExternalOutput").ap()

    with tile.TileContext(nc) as tc, ExitStack() as ctx:
        const = ctx.enter_context(tc.tile_pool(name="const", bufs=1))
        wt_sb = const.tile([P, NCHUNK, E], f32)
        nc.sync.dma_start(wt_sb[:], wt.rearrange("(c p) e -> p c e", p=P))
        bias_bc = const.tile([P, E], f32)
        nc.sync.dma_start(bias_bc[:], bias.to_broadcast((P, E)))
        w_all = const.tile([P, NGRP, 2], f32)
        i_all = const.tile([P, NGRP, 2], i32)

        in_pool = ctx.enter_context(tc.tile_pool(name="hin", bufs=in_bufs))
        sm_pool = ctx.enter_context(tc.tile_pool(name="sm", bufs=3))
        mm_psum = ctx.enter_context(tc.tile_pool(name="mm_psum", bufs=4, space="PSUM"))

        # iters>1: wrap the streaming body in a HW loop for steady-state timing
        with (tc.For_i(0, iters) if iters > 1 else nullcontext()):
            for tt in range(NT):
                htile = in_pool.tile([P, NCHUNK, TOKTILE], f32, tag="hin")
                cpp = NCHUNK // pieces  # chunks per DMA piece
                for piece in range(pieces):
                    j0 = piece * cpp
                    nc.sync.dma_start(
                        htile[:, j0 : j0 + cpp, :], ht5[tt, :, j0 : j0 + cpp, :]
                    )

                lg_pss = [mm_psum.tile([P, E], f32, tag="mmB") for _ in range(GP)]
                if order == "g":
                    # group-major: one 32-chunk accumulation chain at a time
                    mm_iter = [(g, j) for g in range(GP) for j in range(NCHUNK)]
                else:
                    # piece-major: interleave the GP chains so PE lags the
                    # DMA by one piece instead of finishing a tile late
                    mm_iter = [
                        (g, j)
                        for pc in range(pieces)
                        for g in range(GP)
                        for j in range(pc * cpp, (pc + 1) * cpp)
                    ]
                for g, j in mm_iter:
                    nc.tensor.matmul(
                        lg_pss[g][:],
                        lhsT=htile[:, j, g * P : (g + 1) * P],
                        rhs=wt_sb[:, j],
                        start=(j == 0),
                        stop=(j == NCHUNK - 1),
                    )
                lts = []
                for g in range(GP):
                    lt = sm_pool.tile([P, E], f32, tag="ltsb")
                    nc.vector.tensor_add(lt[:], lg_pss[g][:], bias_bc[:])
                    lts.append(lt)

                for g in range(GP):
                    grp = tt * GP + g
                    lt = lts[g]
                    mx8 = sm_pool.tile([P, 8], f32, tag="mx8")
                    nc.vector.max(out=mx8[:], in_=lt[:])
                    idx8 = sm_pool.tile([P, 8], u32, tag="idx8")
                    nc.vector.max_index(idx8[:], mx8[:], lt[:])
                    nc.vector.tensor_copy(i_all[:, grp, :], idx8[:, 0:2])

                    negm = sm_pool.tile([P, 1], f32, tag="negm")
                    nc.vector.tensor_scalar_mul(negm[:], mx8[:, 0:1], -1.0)
                    escr = sm_pool.tile([P, E], f32, tag="escr")
                    zsum = sm_pool.tile([P, 1], f32, tag="zsum")
                    nc.scalar.activation(
                        escr[:],
                        lt[:],
                        mybir.ActivationFunctionType.Exp,
                        bias=negm[:],
                        scale=1.0,
                        accum_out=zsum[:],
                    )
                    e2 = sm_pool.tile([P, 1], f32, tag="e2")
                    nc.scalar.activation(
                        e2[:],
                        mx8[:, 1:2],
                        mybir.ActivationFunctionType.Exp,
                        bias=negm[:],
                        scale=1.0,
                    )
                    zr = sm_pool.tile([P, 1], f32, tag="zr")
                    nc.vector.reciprocal(zr[:], zsum[:])
                    p2 = sm_pool.tile([P, 1], f32, tag="p2")
                    nc.vector.tensor_mul(p2[:], e2[:], zr[:])
                    s = sm_pool.tile([P, 1], f32, tag="s")
                    nc.vector.tensor_add(s[:], zr[:], p2[:])
                    nc.vector.tensor_scalar_add(s[:], s[:], EPS)
                    sr = sm_pool.tile([P, 1], f32, tag="sr")
                    nc.vector.reciprocal(sr[:], s[:])
                    nc.vector.tensor_mul(w_all[:, grp, 0:1], zr[:], sr[:])
                    nc.vector.tensor_mul(w_all[:, grp, 1:2], p2[:], sr[:])

                gsl = slice(tt * GP, (tt + 1) * GP)
                nc.sync.dma_start(w_out[:, gsl, :], w_all[:, gsl, :])
                nc.sync.dma_start(i_out[:, gsl, :], i_all[:, gsl, :])

    nc.compile()
    return nc


def _get_nc():
    global _NC_CACHE
    if _NC_CACHE is None:
        _NC_CACHE = _build_nc()
    return _NC_CACHE


def _make_in_maps(hidden_states, routing_bias, W, b):
    h2 = np.ascontiguousarray(np.asarray(hidden_states, dtype=np.float32)).reshape(
        T_TOTAL, D
    )
    wt = np.ascontiguousarray(np.asarray(W, dtype=np.float32).T)
    bnp = np.asarray(b, dtype=np.float32)
    rb = np.asarray(routing_bias, dtype=np.float32)

    in_maps = []
    for c in range(N_CORES):
        t0 = c * T_CORE
        batch = t0 // S  # each core's token range lies within one batch
        # ht5[tile, p, chunk, tok] = h[t0 + tile*TOKTILE + tok, 128*chunk + p]
        hc = h2[t0 : t0 + T_CORE]  # [T_CORE, D]
        ht5 = np.ascontiguousarray(
            hc.reshape(NT, TOKTILE, NCHUNK, P).transpose(0, 3, 2, 1)
        )
        in_maps.append(
            {
                "ht5": ht5,
                "wt": wt,
                "bias": (bnp + rb[batch]).astype(np.float32).reshape(1, E),
            }
        )
    return in_maps


def kernel(hidden_states, routing_bias, W, b):
    in_maps = _make_in_maps(hidden_states, routing_bias, W, b)
    nc = _get_nc()
    try:
        res = run_bass_kernel_spmd(nc, in_maps, list(range(N_CORES)))
    except Exception:
        # transient NRT/device hiccups have been observed to clear on retry
        res = run_bass_kernel_spmd(nc, in_maps, list(range(N_CORES)))

    ws, idxs = [], []
    for r in res.results:
        # [P, NGRP, 2] with token = grp*128 + partition
        ws.append(np.asarray(r["w_out"]).transpose(1, 0, 2).reshape(T_CORE, 2))
        idxs.append(np.asarray(r["i_out"]).transpose(1, 0, 2).reshape(T_CORE, 2))
    weights = np.concatenate(ws).reshape(B_DIM, S, 2).astype(np.float32)
    indices = np.concatenate(idxs).reshape(B_DIM, S, 2).astype(np.int32)
    return weights, indices
